# revision 19
# baseline (speedup 1.0000x reference)
"""GAT message-passing network (2x GATConv + BN + global mean pool) on 8 trn2 cores.

Two launches (one per GAT layer; layer-1 shard outputs are gathered on the
host between them — the on-device AllGather path crashes NRT through axon).
Sharding: dst-nodes partitioned across cores (6250/core); each core processes
the edges whose dst lands in its shard (edges sorted by dst, padded so every
128-edge tile maps into a single 128-node block). BatchNorms are folded into
the GAT weights on the host (they are affine in eval mode). A dense phase
builds a concatenated node table [x | alo | ahi]; per edge tile the kernel
gathers src/dst rows from it by indirect DMA, computes per-edge features with
a dense matmul, and does segment softmax/aggregation with one-hot
selection-matrix matmuls accumulating in PSUM; the softmax denominator is
divided out after aggregation (it is constant within a segment). Graph-mean
pooling partials come from batch-id one-hot matmuls; the final tiny FC runs
on the host as part of unsharding.
"""

import sys

import numpy as np

for _p in ("/opt/trn_rl_repo",):
    if _p not in sys.path:
        sys.path.insert(0, _p)

import concourse.bass as bass
import concourse.tile as tile
from concourse import bacc, mybir
from concourse.bass import IndirectOffsetOnAxis, ts
from concourse.masks import make_identity

P = 128
FP = mybir.dt.float32
I32 = mybir.dt.int32
AF = mybir.ActivationFunctionType
OP = mybir.AluOpType


def _ceil(a, b):
    return -(-a // b)


def _emit_dense(nc, tc, sfx, x_ap, xc_ap, ww_sb, cb_sb, ident, N, F, H):
    """xcat[n, :] = [x[n] | alo | ahi], alo/ahi = x[n] @ wasad + casad."""
    NT_full = N // P
    GW = 4
    with (
        tc.tile_pool(name=f"dn{sfx}", bufs=3) as dn,
        tc.tile_pool(name=f"dnp{sfx}", bufs=2, space="PSUM") as dnp,
    ):
        groups = [
            list(range(g, min(g + GW, NT_full))) for g in range(0, NT_full, GW)
        ]
        for grp in groups:
            gw = len(grp)
            r0 = grp[0] * P
            xg4 = dn.tile([P, GW * F], FP, tag="xg", name="xg4")
            nc.sync.dma_start(
                xg4[:, : gw * F].rearrange("p (t f) -> p t f", f=F),
                x_ap[r0 : r0 + gw * P, :].rearrange("(t p) f -> p t f", p=P),
            )
            xts4 = dn.tile([F, GW * P], FP, tag="xts", name="xts4")
            for j in range(gw):
                xt_ps = dnp.tile([F, P], FP, tag="t1", space="PSUM", name="xt_ps")
                nc.tensor.transpose(xt_ps[:], xg4[:, j * F : (j + 1) * F], ident[:])
                nc.vector.tensor_copy(xts4[:, j * P : (j + 1) * P], xt_ps[:])
            aa_ps = dnp.tile(
                [2 * H, GW * P], FP, tag="t2", space="PSUM", name="aa_ps"
            )
            nc.tensor.matmul(
                aa_ps[:, : gw * P],
                lhsT=ww_sb[:],
                rhs=xts4[:, : gw * P],
                start=True,
                stop=True,
            )
            aab = dn.tile([2 * H, GW * P], FP, tag="aab", name="aab")
            nc.scalar.activation(
                aab[:, : gw * P], aa_ps[:, : gw * P], AF.Identity, bias=cb_sb[:]
            )
            aaTs = dn.tile([P, GW * 2 * H], FP, tag="aats", name="aaTs")
            for j in range(gw):
                aaT_ps = dnp.tile(
                    [P, 2 * H], FP, tag="t3", space="PSUM", name="aaT_ps"
                )
                nc.tensor.transpose(
                    aaT_ps[:], aab[:, j * P : (j + 1) * P], ident[: 2 * H, : 2 * H]
                )
                nc.vector.tensor_copy(
                    aaTs[:, j * 2 * H : (j + 1) * 2 * H], aaT_ps[:]
                )
            nc.sync.dma_start(
                xc_ap[r0 : r0 + gw * P, :F].rearrange("(t p) f -> p t f", p=P),
                xg4[:, : gw * F].rearrange("p (t f) -> p t f", f=F),
            )
            nc.scalar.dma_start(
                xc_ap[r0 : r0 + gw * P, F:].rearrange("(t p) h -> p t h", p=P),
                aaTs[:, : gw * 2 * H].rearrange("p (t h) -> p t h", h=2 * H),
            )
        if N % P:
            i = NT_full
            p = N - i * P
            xg = dn.tile([P, F], FP, tag="xgp", name="xgp")
            nc.vector.memset(xg[:], 0.0)
            nc.sync.dma_start(xg[:p], x_ap[i * P : i * P + p, :])
            xt_ps = dnp.tile([F, P], FP, tag="t1", space="PSUM", name="xt_psp")
            nc.tensor.transpose(xt_ps[:], xg[:], ident[:])
            xts = dn.tile([F, P], FP, tag="xtsp", name="xtsp")
            nc.vector.tensor_copy(xts[:], xt_ps[:])
            aa_ps = dnp.tile([2 * H, P], FP, tag="t2", space="PSUM", name="aa_psp")
            nc.tensor.matmul(
                aa_ps[:], lhsT=ww_sb[:], rhs=xts[:], start=True, stop=True
            )
            aab = dn.tile([2 * H, P], FP, tag="aabp", name="aabp")
            nc.scalar.activation(aab[:], aa_ps[:], AF.Identity, bias=cb_sb[:])
            aaT_ps = dnp.tile([P, 2 * H], FP, tag="t3", space="PSUM", name="aaT_psp")
            nc.tensor.transpose(aaT_ps[:], aab[:], ident[: 2 * H, : 2 * H])
            aaTs = dn.tile([P, 2 * H], FP, tag="aatsp", name="aaTsp")
            nc.vector.tensor_copy(aaTs[:], aaT_ps[:])
            nc.sync.dma_start(xc_ap[i * P : i * P + p, :F], xg[:p])
            nc.scalar.dma_start(xc_ap[i * P : i * P + p, F:], aaTs[:p])


def _emit_edges(
    nc,
    tc,
    sfx,
    xc_ap,
    ei_ap,
    wp_sb,
    bx_sb,
    ident,
    iotaf,
    F,
    H,
    C,
    NSH,
    TBB,
    NB,
    ho_ap=None,
    pool_cfg=None,
):
    """Edge pass: attention + weighted aggregation per 128-node block.
    Writes block outputs to ho_ap [NSH, C] if given; accumulates graph-pool
    partials if pool_cfg=(bf_ap, po_ap, GCH)."""
    HC = H * C
    hpc = max(1, min(H, 480 // C))
    chunks = [(h0, min(h0 + hpc, H)) for h0 in range(0, H, hpc)]
    EB = (chunks[0][1] - chunks[0][0]) * C
    ycol = [0]
    for k, (h0, h1) in enumerate(chunks):
        ycol.append(ycol[-1] + (h1 - h0) * C + (H if k == 0 else 0))

    with (
        tc.tile_pool(name=f"eg{sfx}", bufs=4) as eg,
        tc.tile_pool(name=f"egp_t{sfx}", bufs=1, space="PSUM") as egp_t,
        tc.tile_pool(name=f"egp_xt{sfx}", bufs=2, space="PSUM") as egp_xt,
        tc.tile_pool(name=f"egp_agg{sfx}", bufs=2, space="PSUM") as egp_agg,
        tc.tile_pool(name=f"egp_pool{sfx}", bufs=1, space="PSUM") as egp_pool,
    ):
        if pool_cfg:
            bf_ap, po_ap, GCH = pool_cfg
            pool_ps = [
                egp_pool.tile(
                    [P, C], FP, tag=f"pool{g}", space="PSUM", name=f"pool{g}"
                )
                for g in range(GCH)
            ]
        cur = None
        offs = [0]
        for v in TBB:
            offs.append(offs[-1] + v)
        for b in range(NB):
          TB = TBB[b]
          for j in range(TB):
            t = offs[b] + j
            et = eg.tile([P, 3], I32, tag="et", name="et")
            nc.scalar.dma_start(et[:], ei_ap[ts(t, P)])
            dlf = eg.tile([P, 1], FP, tag="dlf", name="dlf")
            nc.vector.tensor_copy(dlf[:], et[:, 2:3])
            xsf = eg.tile([P, F + 2 * H], FP, tag="xsf", name="xsf")
            nc.gpsimd.indirect_dma_start(
                out=xsf[:],
                out_offset=None,
                in_=xc_ap[:, :],
                in_offset=IndirectOffsetOnAxis(ap=et[:, 0:1], axis=0),
            )
            gdf = eg.tile([P, F + 2 * H], FP, tag="gdf", name="gdf")
            nc.gpsimd.indirect_dma_start(
                out=gdf[:],
                out_offset=None,
                in_=xc_ap[:, :],
                in_offset=IndirectOffsetOnAxis(ap=et[:, 1:2], axis=0),
            )
            y = eg.tile([P, HC + H], FP, tag="y", name="y")
            pre = eg.tile([P, H], FP, tag="pre", name="pre")
            nc.vector.tensor_tensor(
                out=pre[:],
                in0=xsf[:, F : F + H],
                in1=gdf[:, F + H : F + 2 * H],
                op=OP.add,
            )
            tmp = eg.tile([P, H], FP, tag="tmp", name="tmp")
            nc.vector.tensor_scalar_mul(tmp[:], pre[:], 0.2)
            prl = eg.tile([P, H], FP, tag="prl", name="prl")
            nc.vector.tensor_tensor(prl[:], pre[:], tmp[:], op=OP.max)
            nc.scalar.activation(y[:, EB : EB + H], prl[:], AF.Exp)
            st = eg.tile([P, P], FP, tag="st", name="st")
            nc.vector.tensor_scalar(st[:], iotaf[:], dlf[:, :1], None, OP.is_equal)
            tp = egp_t.tile([F, P], FP, tag="tp", space="PSUM", name="tp")
            nc.tensor.transpose(tp[:], xsf[:, :F], ident[:])
            xts = eg.tile([F, P], FP, tag="xts2", name="xts2")
            nc.vector.tensor_copy(xts[:], tp[:])
            if j == 0:
                cur = [
                    egp_agg.tile(
                        [P, ycol[k + 1] - ycol[k]],
                        FP,
                        tag=f"agg{k}",
                        space="PSUM",
                        name=f"agg{k}",
                    )
                    for k in range(len(chunks))
                ]
            aggs = cur
            for k, (h0, h1) in enumerate(chunks):
                hh = h1 - h0
                xtp = egp_xt.tile(
                    [P, hh * C], FP, tag="xtp", space="PSUM", name="xtp"
                )
                nc.tensor.matmul(
                    xtp[:],
                    lhsT=xts[:],
                    rhs=wp_sb[:, h0 * C : h1 * C],
                    start=True,
                    stop=True,
                )
                yc = ycol[k]
                nc.vector.tensor_tensor(
                    out=y[:, yc : yc + hh * C].rearrange("p (h c) -> p h c", c=C),
                    in0=xtp[:].rearrange("p (h c) -> p h c", c=C),
                    in1=y[:, EB + h0 : EB + h1, None].to_broadcast([P, hh, C]),
                    op=OP.mult,
                )
                nc.tensor.matmul(
                    aggs[k][:],
                    lhsT=st[:],
                    rhs=y[:, ycol[k] : ycol[k + 1]],
                    start=(j == 0),
                    stop=(j == TB - 1),
                )
            if j == TB - 1:
                s1 = eg.tile([P, H], FP, tag="s1", name="s1")
                nc.vector.tensor_scalar_add(s1[:], aggs[0][:, EB : EB + H], 1e-16)
                siv = eg.tile([P, H], FP, tag="siv", name="siv")
                nc.vector.reciprocal(siv[:], s1[:])
                t1 = eg.tile([P, HC], FP, tag="t1f", name="t1f")
                for k, (h0, h1) in enumerate(chunks):
                    hh = h1 - h0
                    nc.vector.tensor_tensor(
                        out=t1[:, h0 * C : h1 * C].rearrange(
                            "p (h c) -> p h c", c=C
                        ),
                        in0=aggs[k][:, : hh * C].rearrange("p (h c) -> p h c", c=C),
                        in1=siv[:, h0:h1, None].to_broadcast([P, hh, C]),
                        op=OP.mult,
                    )
                t2 = eg.tile([P, C], FP, tag="t2f", name="t2f")
                nc.vector.reduce_sum(
                    t2[:, :, None],
                    t1[:].rearrange("p (h c) -> p c h", c=C),
                    axis=mybir.AxisListType.X,
                )
                t3 = eg.tile([P, C], FP, tag="t3f", name="t3f")
                nc.vector.tensor_tensor(t3[:], t2[:], bx_sb[:], op=OP.add)
                u = eg.tile([P, C], FP, tag="uf", name="uf")
                nc.vector.tensor_scalar_mul(u[:], t3[:], 1.0 / H)
                v = eg.tile([P, C], FP, tag="vf", name="vf")
                nc.vector.tensor_scalar_mul(v[:], u[:], 0.01)
                ho = eg.tile([P, C], FP, tag="hof", name="hof")
                nc.vector.tensor_tensor(ho[:], u[:], v[:], op=OP.max)
                valid = min(P, NSH - b * P)
                if ho_ap is not None:
                    nc.scalar.dma_start(ho_ap[b * P : b * P + valid, :], ho[:valid])
                if pool_cfg:
                    bf = eg.tile([P, 1], FP, tag="bf", name="bf")
                    nc.sync.dma_start(bf[:], bf_ap[ts(b, P)])
                    for g in range(GCH):
                        bs = eg.tile([P, 1], FP, tag="bs", name="bs")
                        nc.vector.tensor_scalar(
                            bs[:], bf[:], float(g * P), None, OP.subtract
                        )
                        sb = eg.tile([P, P], FP, tag="sb", name="sb")
                        nc.vector.tensor_scalar(
                            sb[:], iotaf[:], bs[:, :1], None, OP.is_equal
                        )
                        nc.tensor.matmul(
                            pool_ps[g][:],
                            lhsT=sb[:],
                            rhs=ho[:],
                            start=(b == 0),
                            stop=(b == NB - 1),
                        )
        if pool_cfg:
            for g in range(GCH):
                pc = eg.tile([P, C], FP, tag="pc", name="pc")
                nc.vector.tensor_copy(pc[:], pool_ps[g][:])
                nc.sync.dma_start(po_ap[ts(g, P)], pc[:])


def build_layer_program(N, F, H, C, NSH, TBB, NB, Ep, G=None):
    """One GAT layer (+ optional graph pooling) over a dst-node shard."""
    pool = G is not None
    GCH = _ceil(G, P) if pool else 0
    nc = bacc.Bacc("TRN2", target_bir_lowering=False, debug=False)

    x_d = nc.dram_tensor("x", [N, F], FP, kind="ExternalInput").ap()
    wp_d = nc.dram_tensor("wp", [F, H * C], FP, kind="ExternalInput").ap()
    ww_d = nc.dram_tensor("wasad", [F, 2 * H], FP, kind="ExternalInput").ap()
    cb_d = nc.dram_tensor("casad", [2 * H, 1], FP, kind="ExternalInput").ap()
    bx_d = nc.dram_tensor("biasx", [P, C], FP, kind="ExternalInput").ap()
    ei_d = nc.dram_tensor("eidx", [Ep, 3], I32, kind="ExternalInput").ap()
    if pool:
        bf_d = nc.dram_tensor("batchf", [NB * P, 1], FP, kind="ExternalInput").ap()
        po_d = nc.dram_tensor("pooled", [GCH * P, C], FP, kind="ExternalOutput").ap()
    ho_d = nc.dram_tensor("hout", [NSH, C], FP, kind="ExternalOutput").ap()
    xc_d = nc.dram_tensor("xcat", [N, F + 2 * H], FP).ap()

    with tile.TileContext(nc) as tc:
        with tc.tile_pool(name="const", bufs=1) as const:
            ident = const.tile([P, P], FP)
            make_identity(nc, ident[:])
            iota_i = const.tile([P, P], I32)
            nc.gpsimd.iota(iota_i[:], pattern=[[1, P]], base=0, channel_multiplier=0)
            iotaf = const.tile([P, P], FP)
            nc.vector.tensor_copy(iotaf[:], iota_i[:])

            def load_const(name, ap_, shape):
                sb = const.tile(shape, FP, name=name)
                nc.sync.dma_start(sb[:], ap_[:, :])
                return sb

            wp_sb = load_const("wp_sb", wp_d, [F, H * C])
            ww_sb = load_const("ww_sb", ww_d, [F, 2 * H])
            cb_sb = load_const("cb_sb", cb_d, [2 * H, 1])
            bx_sb = load_const("bx_sb", bx_d, [P, C])

            _emit_dense(nc, tc, "1", x_d, xc_d, ww_sb, cb_sb, ident, N, F, H)
            _emit_edges(
                nc, tc, "1", xc_d, ei_d, wp_sb, bx_sb, ident, iotaf,
                F, H, C, NSH, TBB, NB, ho_ap=ho_d,
                pool_cfg=(bf_d, po_d, GCH) if pool else None,
            )
    nc.compile()
    return nc


def _fold(W, a_s, a_d, b, g, bb, m, v, H, C):
    """Fold eval-mode BatchNorm (scale s, shift t) into GAT weights."""
    F = W.shape[0]
    s = g / np.sqrt(v + 1e-5)
    t = bb - m * s
    Wp = (s[:, None] * W).astype(np.float32)
    cxt = t @ W  # [H*C]
    Wr = Wp.reshape(F, H, C)
    was = np.einsum("fhc,hc->fh", Wr, a_s)
    wad = np.einsum("fhc,hc->fh", Wr, a_d)
    cas = np.einsum("hc,hc->h", cxt.reshape(H, C), a_s)
    cad = np.einsum("hc,hc->h", cxt.reshape(H, C), a_d)
    bias_true = b + cxt.reshape(H, C).mean(0)  # [C]
    wasad = np.concatenate([was, wad], 1).astype(np.float32)
    casad = np.concatenate([cas, cad])[:, None].astype(np.float32)
    biasx = np.broadcast_to(H * bias_true, (P, C)).astype(np.float32).copy()
    return Wp, wasad, casad, biasx


def _prep_edges(src, dst, N, NC, NSH):
    """Sort edges by dst, shard by dst-node, pad each 128-node block to a
    uniform number of 128-edge tiles (TB). Pads have dstloc=200 -> zero
    one-hot column -> inert. Returns eidx [NC, Ep, 3] int32 =
    (src row, dst row, block-local dst)."""
    order = np.argsort(dst, kind="stable")
    ss = src[order].astype(np.int64)
    ds = dst[order].astype(np.int64)
    core = ds // NSH
    loc = ds - core * NSH
    NB = _ceil(NSH, P)
    blk = loc // P
    gid = core * NB + blk
    cnt = np.bincount(gid, minlength=NC * NB)
    # per-block tile count: max over cores for that block (SPMD-uniform)
    tbb = np.maximum(1, -(-cnt.reshape(NC, NB).max(axis=0) // P))
    offs = np.zeros(NB + 1, np.int64)
    offs[1:] = np.cumsum(tbb)
    Ep = int(offs[NB]) * P
    starts = np.zeros(NC * NB, np.int64)
    starts[1:] = np.cumsum(cnt)[:-1]
    rank = np.arange(len(ds)) - starts[gid]
    pos = offs[blk] * P + rank
    eidx = np.zeros((NC, Ep, 3), np.int32)
    eidx[:, :, 2] = 200
    eidx[core, pos, 0] = ss
    eidx[core, pos, 1] = ds
    eidx[core, pos, 2] = (loc - blk * P).astype(np.int32)
    return eidx, [int(v) for v in tbb], NB, Ep


TRACE = False  # test harness sets True to collect HW exec times
LAST_EXEC_NS = []
LAST_PROGRAMS = []  # (nc, in_maps) pairs from the most recent kernel() call


def _run(nc, in_maps, out_names, sim=False):
    if sim:
        from concourse.bass_interp import CoreSim, MultiCoreSim

        ncores = len(in_maps)
        if nc.has_collectives:
            ms = MultiCoreSim(nc, ncores)
            for c, m in enumerate(in_maps):
                for k, val in m.items():
                    ms.cores[c].tensor(k)[:] = val
            ms.simulate()
            return [
                {o: np.array(ms.cores[c].mem_tensor(o)) for o in out_names}
                for c in range(ncores)
            ]
        res = []
        for m in in_maps:
            s = CoreSim(nc)
            for k, val in m.items():
                s.tensor(k)[:] = val
            s.simulate()
            res.append({o: np.array(s.mem_tensor(o)) for o in out_names})
        return res
    from concourse.bass_utils import run_bass_kernel_spmd

    LAST_PROGRAMS.append((nc, in_maps))
    r = run_bass_kernel_spmd(
        nc, in_maps, core_ids=list(range(len(in_maps))), trace=TRACE
    )
    if TRACE:
        LAST_EXEC_NS.append(r.exec_time_ns)
    return r.results


def kernel(
    x,
    edge_index,
    batch,
    bn1_g,
    bn1_b,
    bn1_m,
    bn1_v,
    W1,
    as1,
    ad1,
    b1,
    bn2_g,
    bn2_b,
    bn2_m,
    bn2_v,
    W2,
    as2,
    ad2,
    b2,
    fc_w,
    fc_b,
    _n_cores=8,
    _G=256,
    _sim=False,
):
    x = np.ascontiguousarray(np.asarray(x, np.float32))
    ei = np.asarray(edge_index).astype(np.int64)
    batch = np.asarray(batch).astype(np.int64)
    to32 = lambda a: np.ascontiguousarray(np.asarray(a, np.float32))
    W1, as1, ad1, b1 = to32(W1), to32(as1), to32(ad1), to32(b1)
    W2, as2, ad2, b2 = to32(W2), to32(as2), to32(ad2), to32(b2)
    fc_w, fc_b = to32(fc_w), to32(fc_b)

    N, F = x.shape
    H, C1 = as1.shape
    C2 = as2.shape[1]
    NC = _n_cores
    G = _G
    NSH = N // NC
    assert NSH * NC == N

    src = np.concatenate([ei[0], np.arange(N, dtype=np.int64)])
    dst = np.concatenate([ei[1], np.arange(N, dtype=np.int64)])
    eidx, TBB, NB, Ep = _prep_edges(src, dst, N, NC, NSH)

    Wp1, wasad1, casad1, biasx1 = _fold(
        W1, as1, ad1, b1, bn1_g, bn1_b, bn1_m, bn1_v, H, C1
    )
    Wp2, wasad2, casad2, biasx2 = _fold(
        W2, as2, ad2, b2, bn2_g, bn2_b, bn2_m, bn2_v, H, C2
    )

    # ---- launch 1: GAT layer 1 -> h2 shards ----
    nc1 = build_layer_program(N, F, H, C1, NSH, TBB, NB, Ep)
    maps1 = [
        {
            "x": x,
            "wp": Wp1,
            "wasad": wasad1,
            "casad": casad1,
            "biasx": biasx1,
            "eidx": eidx[c],
        }
        for c in range(NC)
    ]
    r1 = _run(nc1, maps1, ["hout"], sim=_sim)
    h2 = np.concatenate([r1[c]["hout"] for c in range(NC)], 0)

    # ---- launch 2: GAT layer 2 + graph pooling partials ----
    nc2 = build_layer_program(N, C1, H, C2, NSH, TBB, NB, Ep, G=G)
    maps2 = []
    for c in range(NC):
        bf = np.full((NB * P, 1), 1e4, np.float32)
        bf[:NSH, 0] = batch[c * NSH : (c + 1) * NSH]
        maps2.append(
            {
                "x": h2,
                "wp": Wp2,
                "wasad": wasad2,
                "casad": casad2,
                "biasx": biasx2,
                "eidx": eidx[c],
                "batchf": bf,
            }
        )
    r2 = _run(nc2, maps2, ["pooled"], sim=_sim)

    pool_sum = np.sum([r2[c]["pooled"] for c in range(NC)], axis=0)[:G]
    cnt = np.bincount(batch, minlength=G).astype(np.float32)
    pooled = pool_sum / np.maximum(cnt, 1.0)[:, None]
    return (pooled @ fc_w + fc_b).astype(np.float32)


# revision 20
# speedup vs baseline: 1.1241x; 1.1241x over previous
"""GAT message-passing network (2x GATConv + BN + global mean pool) on 8 trn2 cores.

Two launches (one per GAT layer; layer-1 shard outputs are gathered on the
host between them — the on-device AllGather path crashes NRT through axon).
Sharding: dst-nodes partitioned across cores (6250/core); each core processes
the edges whose dst lands in its shard (edges sorted by dst, padded so every
128-edge tile maps into a single 128-node block). BatchNorms are folded into
the GAT weights on the host (they are affine in eval mode). A dense phase
builds a concatenated node table [x | alo | ahi]; per edge tile the kernel
gathers src/dst rows from it by indirect DMA, computes per-edge features with
a dense matmul, and does segment softmax/aggregation with one-hot
selection-matrix matmuls accumulating in PSUM; the softmax denominator is
divided out after aggregation (it is constant within a segment). Graph-mean
pooling partials come from batch-id one-hot matmuls; the final tiny FC runs
on the host as part of unsharding.
"""

import sys

import numpy as np

for _p in ("/opt/trn_rl_repo",):
    if _p not in sys.path:
        sys.path.insert(0, _p)

import concourse.bass as bass
import concourse.tile as tile
from concourse import bacc, mybir
from concourse.bass import IndirectOffsetOnAxis, ts
from concourse.masks import make_identity

P = 128
FP = mybir.dt.float32
I32 = mybir.dt.int32
AF = mybir.ActivationFunctionType
OP = mybir.AluOpType


def _ceil(a, b):
    return -(-a // b)


def _emit_dense(nc, tc, sfx, x_ap, xc_ap, ww_sb, cb_sb, ident, N, F, H):
    """xcat[n, :] = [x[n] | alo | ahi], alo/ahi = x[n] @ wasad + casad."""
    NT_full = N // P
    GW = 4
    with (
        tc.tile_pool(name=f"dn{sfx}", bufs=3) as dn,
        tc.tile_pool(name=f"dnp{sfx}", bufs=2, space="PSUM") as dnp,
    ):
        groups = [
            list(range(g, min(g + GW, NT_full))) for g in range(0, NT_full, GW)
        ]
        for grp in groups:
            gw = len(grp)
            r0 = grp[0] * P
            xg4 = dn.tile([P, GW * F], FP, tag="xg", name="xg4")
            nc.sync.dma_start(
                xg4[:, : gw * F].rearrange("p (t f) -> p t f", f=F),
                x_ap[r0 : r0 + gw * P, :].rearrange("(t p) f -> p t f", p=P),
            )
            xts4 = dn.tile([F, GW * P], FP, tag="xts", name="xts4")
            xt_ps4 = dnp.tile([F, GW * P], FP, tag="t1", space="PSUM", name="xt_ps4")
            for j in range(gw):
                nc.tensor.transpose(
                    xt_ps4[:, j * P : (j + 1) * P],
                    xg4[:, j * F : (j + 1) * F],
                    ident[:],
                )
            nc.scalar.copy(xts4[:, : gw * P], xt_ps4[:, : gw * P])
            aa_ps = dnp.tile(
                [2 * H, GW * P], FP, tag="t2", space="PSUM", name="aa_ps"
            )
            nc.tensor.matmul(
                aa_ps[:, : gw * P],
                lhsT=ww_sb[:],
                rhs=xts4[:, : gw * P],
                start=True,
                stop=True,
            )
            aab = dn.tile([2 * H, GW * P], FP, tag="aab", name="aab")
            nc.scalar.activation(
                aab[:, : gw * P], aa_ps[:, : gw * P], AF.Identity, bias=cb_sb[:]
            )
            aaTs = dn.tile([P, GW * 2 * H], FP, tag="aats", name="aaTs")
            aaT_ps4 = dnp.tile(
                [P, GW * 2 * H], FP, tag="t3", space="PSUM", name="aaT_ps4"
            )
            for j in range(gw):
                nc.tensor.transpose(
                    aaT_ps4[:, j * 2 * H : (j + 1) * 2 * H],
                    aab[:, j * P : (j + 1) * P],
                    ident[: 2 * H, : 2 * H],
                )
            nc.vector.tensor_copy(aaTs[:, : gw * 2 * H], aaT_ps4[:, : gw * 2 * H])
            nc.sync.dma_start(
                xc_ap[r0 : r0 + gw * P, :F].rearrange("(t p) f -> p t f", p=P),
                xg4[:, : gw * F].rearrange("p (t f) -> p t f", f=F),
            )
            nc.scalar.dma_start(
                xc_ap[r0 : r0 + gw * P, F:].rearrange("(t p) h -> p t h", p=P),
                aaTs[:, : gw * 2 * H].rearrange("p (t h) -> p t h", h=2 * H),
            )
        if N % P:
            i = NT_full
            p = N - i * P
            xg = dn.tile([P, F], FP, tag="xgp", name="xgp")
            nc.vector.memset(xg[:], 0.0)
            nc.sync.dma_start(xg[:p], x_ap[i * P : i * P + p, :])
            xt_ps = dnp.tile([F, P], FP, tag="t1", space="PSUM", name="xt_psp")
            nc.tensor.transpose(xt_ps[:], xg[:], ident[:])
            xts = dn.tile([F, P], FP, tag="xtsp", name="xtsp")
            nc.vector.tensor_copy(xts[:], xt_ps[:])
            aa_ps = dnp.tile([2 * H, P], FP, tag="t2", space="PSUM", name="aa_psp")
            nc.tensor.matmul(
                aa_ps[:], lhsT=ww_sb[:], rhs=xts[:], start=True, stop=True
            )
            aab = dn.tile([2 * H, P], FP, tag="aabp", name="aabp")
            nc.scalar.activation(aab[:], aa_ps[:], AF.Identity, bias=cb_sb[:])
            aaT_ps = dnp.tile([P, 2 * H], FP, tag="t3", space="PSUM", name="aaT_psp")
            nc.tensor.transpose(aaT_ps[:], aab[:], ident[: 2 * H, : 2 * H])
            aaTs = dn.tile([P, 2 * H], FP, tag="aatsp", name="aaTsp")
            nc.vector.tensor_copy(aaTs[:], aaT_ps[:])
            nc.sync.dma_start(xc_ap[i * P : i * P + p, :F], xg[:p])
            nc.scalar.dma_start(xc_ap[i * P : i * P + p, F:], aaTs[:p])


def _emit_edges(
    nc,
    tc,
    sfx,
    xc_ap,
    ei_ap,
    wp_sb,
    bx_sb,
    ident,
    iotaf,
    F,
    H,
    C,
    NSH,
    TBB,
    NB,
    ho_ap=None,
    pool_cfg=None,
):
    """Edge pass: attention + weighted aggregation per 128-node block.
    Writes block outputs to ho_ap [NSH, C] if given; accumulates graph-pool
    partials if pool_cfg=(bf_ap, po_ap, GCH)."""
    HC = H * C
    hpc = max(1, min(H, 480 // C))
    chunks = [(h0, min(h0 + hpc, H)) for h0 in range(0, H, hpc)]
    EB = (chunks[0][1] - chunks[0][0]) * C
    ycol = [0]
    for k, (h0, h1) in enumerate(chunks):
        ycol.append(ycol[-1] + (h1 - h0) * C + (H if k == 0 else 0))

    with (
        tc.tile_pool(name=f"eg{sfx}", bufs=4) as eg,
        tc.tile_pool(name=f"egp_t{sfx}", bufs=1, space="PSUM") as egp_t,
        tc.tile_pool(name=f"egp_xt{sfx}", bufs=2, space="PSUM") as egp_xt,
        tc.tile_pool(name=f"egp_agg{sfx}", bufs=2, space="PSUM") as egp_agg,
        tc.tile_pool(name=f"egp_pool{sfx}", bufs=1, space="PSUM") as egp_pool,
    ):
        if pool_cfg:
            bf_ap, po_ap, GCH = pool_cfg
            pool_ps = [
                egp_pool.tile(
                    [P, C], FP, tag=f"pool{g}", space="PSUM", name=f"pool{g}"
                )
                for g in range(GCH)
            ]
        cur = None
        offs = [0]
        for v in TBB:
            offs.append(offs[-1] + v)
        for b in range(NB):
          TB = TBB[b]
          for j in range(TB):
            t = offs[b] + j
            et = eg.tile([P, 3], I32, tag="et", name="et")
            nc.scalar.dma_start(et[:], ei_ap[ts(t, P)])
            dlf = eg.tile([P, 1], FP, tag="dlf", name="dlf")
            nc.vector.tensor_copy(dlf[:], et[:, 2:3])
            xsf = eg.tile([P, F + 2 * H], FP, tag="xsf", name="xsf")
            nc.gpsimd.indirect_dma_start(
                out=xsf[:],
                out_offset=None,
                in_=xc_ap[:, :],
                in_offset=IndirectOffsetOnAxis(ap=et[:, 0:1], axis=0),
            )
            gdf = eg.tile([P, F + 2 * H], FP, tag="gdf", name="gdf")
            nc.gpsimd.indirect_dma_start(
                out=gdf[:],
                out_offset=None,
                in_=xc_ap[:, :],
                in_offset=IndirectOffsetOnAxis(ap=et[:, 1:2], axis=0),
            )
            y = eg.tile([P, HC + H], FP, tag="y", name="y")
            pre = eg.tile([P, H], FP, tag="pre", name="pre")
            nc.vector.tensor_tensor(
                out=pre[:],
                in0=xsf[:, F : F + H],
                in1=gdf[:, F + H : F + 2 * H],
                op=OP.add,
            )
            tmp = eg.tile([P, H], FP, tag="tmp", name="tmp")
            nc.vector.tensor_scalar_mul(tmp[:], pre[:], 0.2)
            prl = eg.tile([P, H], FP, tag="prl", name="prl")
            nc.vector.tensor_tensor(prl[:], pre[:], tmp[:], op=OP.max)
            nc.scalar.activation(y[:, EB : EB + H], prl[:], AF.Exp)
            st = eg.tile([P, P], FP, tag="st", name="st")
            nc.vector.tensor_scalar(st[:], iotaf[:], dlf[:, :1], None, OP.is_equal)
            tp = egp_t.tile([F, P], FP, tag="tp", space="PSUM", name="tp")
            nc.tensor.transpose(tp[:], xsf[:, :F], ident[:])
            xts = eg.tile([F, P], FP, tag="xts2", name="xts2")
            nc.vector.tensor_copy(xts[:], tp[:])
            if j == 0:
                cur = [
                    egp_agg.tile(
                        [P, ycol[k + 1] - ycol[k]],
                        FP,
                        tag=f"agg{k}",
                        space="PSUM",
                        name=f"agg{k}",
                    )
                    for k in range(len(chunks))
                ]
            aggs = cur
            for k, (h0, h1) in enumerate(chunks):
                hh = h1 - h0
                xtp = egp_xt.tile(
                    [P, hh * C], FP, tag="xtp", space="PSUM", name="xtp"
                )
                nc.tensor.matmul(
                    xtp[:],
                    lhsT=xts[:],
                    rhs=wp_sb[:, h0 * C : h1 * C],
                    start=True,
                    stop=True,
                )
                yc = ycol[k]
                nc.vector.tensor_tensor(
                    out=y[:, yc : yc + hh * C].rearrange("p (h c) -> p h c", c=C),
                    in0=xtp[:].rearrange("p (h c) -> p h c", c=C),
                    in1=y[:, EB + h0 : EB + h1, None].to_broadcast([P, hh, C]),
                    op=OP.mult,
                )
                nc.tensor.matmul(
                    aggs[k][:],
                    lhsT=st[:],
                    rhs=y[:, ycol[k] : ycol[k + 1]],
                    start=(j == 0),
                    stop=(j == TB - 1),
                )
            if j == TB - 1:
                s1 = eg.tile([P, H], FP, tag="s1", name="s1")
                nc.vector.tensor_scalar_add(s1[:], aggs[0][:, EB : EB + H], 1e-16)
                siv = eg.tile([P, H], FP, tag="siv", name="siv")
                nc.vector.reciprocal(siv[:], s1[:])
                t1 = eg.tile([P, HC], FP, tag="t1f", name="t1f")
                for k, (h0, h1) in enumerate(chunks):
                    hh = h1 - h0
                    nc.vector.tensor_tensor(
                        out=t1[:, h0 * C : h1 * C].rearrange(
                            "p (h c) -> p h c", c=C
                        ),
                        in0=aggs[k][:, : hh * C].rearrange("p (h c) -> p h c", c=C),
                        in1=siv[:, h0:h1, None].to_broadcast([P, hh, C]),
                        op=OP.mult,
                    )
                t2 = eg.tile([P, C], FP, tag="t2f", name="t2f")
                nc.vector.reduce_sum(
                    t2[:, :, None],
                    t1[:].rearrange("p (h c) -> p c h", c=C),
                    axis=mybir.AxisListType.X,
                )
                t3 = eg.tile([P, C], FP, tag="t3f", name="t3f")
                nc.vector.tensor_tensor(t3[:], t2[:], bx_sb[:], op=OP.add)
                u = eg.tile([P, C], FP, tag="uf", name="uf")
                nc.vector.tensor_scalar_mul(u[:], t3[:], 1.0 / H)
                v = eg.tile([P, C], FP, tag="vf", name="vf")
                nc.vector.tensor_scalar_mul(v[:], u[:], 0.01)
                ho = eg.tile([P, C], FP, tag="hof", name="hof")
                nc.vector.tensor_tensor(ho[:], u[:], v[:], op=OP.max)
                valid = min(P, NSH - b * P)
                if ho_ap is not None:
                    nc.scalar.dma_start(ho_ap[b * P : b * P + valid, :], ho[:valid])
                if pool_cfg:
                    bf = eg.tile([P, 1], FP, tag="bf", name="bf")
                    nc.sync.dma_start(bf[:], bf_ap[ts(b, P)])
                    for g in range(GCH):
                        bs = eg.tile([P, 1], FP, tag="bs", name="bs")
                        nc.vector.tensor_scalar(
                            bs[:], bf[:], float(g * P), None, OP.subtract
                        )
                        sb = eg.tile([P, P], FP, tag="sb", name="sb")
                        nc.vector.tensor_scalar(
                            sb[:], iotaf[:], bs[:, :1], None, OP.is_equal
                        )
                        nc.tensor.matmul(
                            pool_ps[g][:],
                            lhsT=sb[:],
                            rhs=ho[:],
                            start=(b == 0),
                            stop=(b == NB - 1),
                        )
        if pool_cfg:
            for g in range(GCH):
                pc = eg.tile([P, C], FP, tag="pc", name="pc")
                nc.vector.tensor_copy(pc[:], pool_ps[g][:])
                nc.sync.dma_start(po_ap[ts(g, P)], pc[:])


def build_layer_program(N, F, H, C, NSH, TBB, NB, Ep, G=None):
    """One GAT layer (+ optional graph pooling) over a dst-node shard."""
    pool = G is not None
    GCH = _ceil(G, P) if pool else 0
    nc = bacc.Bacc("TRN2", target_bir_lowering=False, debug=False)

    x_d = nc.dram_tensor("x", [N, F], FP, kind="ExternalInput").ap()
    wp_d = nc.dram_tensor("wp", [F, H * C], FP, kind="ExternalInput").ap()
    ww_d = nc.dram_tensor("wasad", [F, 2 * H], FP, kind="ExternalInput").ap()
    cb_d = nc.dram_tensor("casad", [2 * H, 1], FP, kind="ExternalInput").ap()
    bx_d = nc.dram_tensor("biasx", [P, C], FP, kind="ExternalInput").ap()
    ei_d = nc.dram_tensor("eidx", [Ep, 3], I32, kind="ExternalInput").ap()
    if pool:
        bf_d = nc.dram_tensor("batchf", [NB * P, 1], FP, kind="ExternalInput").ap()
        po_d = nc.dram_tensor("pooled", [GCH * P, C], FP, kind="ExternalOutput").ap()
    ho_d = nc.dram_tensor("hout", [NSH, C], FP, kind="ExternalOutput").ap()
    xc_d = nc.dram_tensor("xcat", [N, F + 2 * H], FP).ap()

    with tile.TileContext(nc) as tc:
        with tc.tile_pool(name="const", bufs=1) as const:
            ident = const.tile([P, P], FP)
            make_identity(nc, ident[:])
            iota_i = const.tile([P, P], I32)
            nc.gpsimd.iota(iota_i[:], pattern=[[1, P]], base=0, channel_multiplier=0)
            iotaf = const.tile([P, P], FP)
            nc.vector.tensor_copy(iotaf[:], iota_i[:])

            def load_const(name, ap_, shape):
                sb = const.tile(shape, FP, name=name)
                nc.sync.dma_start(sb[:], ap_[:, :])
                return sb

            wp_sb = load_const("wp_sb", wp_d, [F, H * C])
            ww_sb = load_const("ww_sb", ww_d, [F, 2 * H])
            cb_sb = load_const("cb_sb", cb_d, [2 * H, 1])
            bx_sb = load_const("bx_sb", bx_d, [P, C])

            _emit_dense(nc, tc, "1", x_d, xc_d, ww_sb, cb_sb, ident, N, F, H)
            _emit_edges(
                nc, tc, "1", xc_d, ei_d, wp_sb, bx_sb, ident, iotaf,
                F, H, C, NSH, TBB, NB, ho_ap=ho_d,
                pool_cfg=(bf_d, po_d, GCH) if pool else None,
            )
    nc.compile()
    return nc


def _fold(W, a_s, a_d, b, g, bb, m, v, H, C):
    """Fold eval-mode BatchNorm (scale s, shift t) into GAT weights."""
    F = W.shape[0]
    s = g / np.sqrt(v + 1e-5)
    t = bb - m * s
    Wp = (s[:, None] * W).astype(np.float32)
    cxt = t @ W  # [H*C]
    Wr = Wp.reshape(F, H, C)
    was = np.einsum("fhc,hc->fh", Wr, a_s)
    wad = np.einsum("fhc,hc->fh", Wr, a_d)
    cas = np.einsum("hc,hc->h", cxt.reshape(H, C), a_s)
    cad = np.einsum("hc,hc->h", cxt.reshape(H, C), a_d)
    bias_true = b + cxt.reshape(H, C).mean(0)  # [C]
    wasad = np.concatenate([was, wad], 1).astype(np.float32)
    casad = np.concatenate([cas, cad])[:, None].astype(np.float32)
    biasx = np.broadcast_to(H * bias_true, (P, C)).astype(np.float32).copy()
    return Wp, wasad, casad, biasx


def _prep_edges(src, dst, N, NC, NSH):
    """Sort edges by dst, shard by dst-node, pad each 128-node block to a
    uniform number of 128-edge tiles (TB). Pads have dstloc=200 -> zero
    one-hot column -> inert. Returns eidx [NC, Ep, 3] int32 =
    (src row, dst row, block-local dst)."""
    order = np.argsort(dst, kind="stable")
    ss = src[order].astype(np.int64)
    ds = dst[order].astype(np.int64)
    core = ds // NSH
    loc = ds - core * NSH
    NB = _ceil(NSH, P)
    blk = loc // P
    gid = core * NB + blk
    cnt = np.bincount(gid, minlength=NC * NB)
    # per-block tile count: max over cores for that block (SPMD-uniform)
    tbb = np.maximum(1, -(-cnt.reshape(NC, NB).max(axis=0) // P))
    offs = np.zeros(NB + 1, np.int64)
    offs[1:] = np.cumsum(tbb)
    Ep = int(offs[NB]) * P
    starts = np.zeros(NC * NB, np.int64)
    starts[1:] = np.cumsum(cnt)[:-1]
    rank = np.arange(len(ds)) - starts[gid]
    pos = offs[blk] * P + rank
    eidx = np.zeros((NC, Ep, 3), np.int32)
    eidx[:, :, 2] = 200
    eidx[core, pos, 0] = ss
    eidx[core, pos, 1] = ds
    eidx[core, pos, 2] = (loc - blk * P).astype(np.int32)
    return eidx, [int(v) for v in tbb], NB, Ep


TRACE = False  # test harness sets True to collect HW exec times
LAST_EXEC_NS = []
LAST_PROGRAMS = []  # (nc, in_maps) pairs from the most recent kernel() call


def _run(nc, in_maps, out_names, sim=False):
    if sim:
        from concourse.bass_interp import CoreSim, MultiCoreSim

        ncores = len(in_maps)
        if nc.has_collectives:
            ms = MultiCoreSim(nc, ncores)
            for c, m in enumerate(in_maps):
                for k, val in m.items():
                    ms.cores[c].tensor(k)[:] = val
            ms.simulate()
            return [
                {o: np.array(ms.cores[c].mem_tensor(o)) for o in out_names}
                for c in range(ncores)
            ]
        res = []
        for m in in_maps:
            s = CoreSim(nc)
            for k, val in m.items():
                s.tensor(k)[:] = val
            s.simulate()
            res.append({o: np.array(s.mem_tensor(o)) for o in out_names})
        return res
    from concourse.bass_utils import run_bass_kernel_spmd

    LAST_PROGRAMS.append((nc, in_maps))
    r = run_bass_kernel_spmd(
        nc, in_maps, core_ids=list(range(len(in_maps))), trace=TRACE
    )
    if TRACE:
        LAST_EXEC_NS.append(r.exec_time_ns)
    return r.results


def kernel(
    x,
    edge_index,
    batch,
    bn1_g,
    bn1_b,
    bn1_m,
    bn1_v,
    W1,
    as1,
    ad1,
    b1,
    bn2_g,
    bn2_b,
    bn2_m,
    bn2_v,
    W2,
    as2,
    ad2,
    b2,
    fc_w,
    fc_b,
    _n_cores=8,
    _G=256,
    _sim=False,
):
    x = np.ascontiguousarray(np.asarray(x, np.float32))
    ei = np.asarray(edge_index).astype(np.int64)
    batch = np.asarray(batch).astype(np.int64)
    to32 = lambda a: np.ascontiguousarray(np.asarray(a, np.float32))
    W1, as1, ad1, b1 = to32(W1), to32(as1), to32(ad1), to32(b1)
    W2, as2, ad2, b2 = to32(W2), to32(as2), to32(ad2), to32(b2)
    fc_w, fc_b = to32(fc_w), to32(fc_b)

    N, F = x.shape
    H, C1 = as1.shape
    C2 = as2.shape[1]
    NC = _n_cores
    G = _G
    NSH = N // NC
    assert NSH * NC == N

    src = np.concatenate([ei[0], np.arange(N, dtype=np.int64)])
    dst = np.concatenate([ei[1], np.arange(N, dtype=np.int64)])
    eidx, TBB, NB, Ep = _prep_edges(src, dst, N, NC, NSH)

    Wp1, wasad1, casad1, biasx1 = _fold(
        W1, as1, ad1, b1, bn1_g, bn1_b, bn1_m, bn1_v, H, C1
    )
    Wp2, wasad2, casad2, biasx2 = _fold(
        W2, as2, ad2, b2, bn2_g, bn2_b, bn2_m, bn2_v, H, C2
    )

    # ---- launch 1: GAT layer 1 -> h2 shards ----
    nc1 = build_layer_program(N, F, H, C1, NSH, TBB, NB, Ep)
    maps1 = [
        {
            "x": x,
            "wp": Wp1,
            "wasad": wasad1,
            "casad": casad1,
            "biasx": biasx1,
            "eidx": eidx[c],
        }
        for c in range(NC)
    ]
    r1 = _run(nc1, maps1, ["hout"], sim=_sim)
    h2 = np.concatenate([r1[c]["hout"] for c in range(NC)], 0)

    # ---- launch 2: GAT layer 2 + graph pooling partials ----
    nc2 = build_layer_program(N, C1, H, C2, NSH, TBB, NB, Ep, G=G)
    maps2 = []
    for c in range(NC):
        bf = np.full((NB * P, 1), 1e4, np.float32)
        bf[:NSH, 0] = batch[c * NSH : (c + 1) * NSH]
        maps2.append(
            {
                "x": h2,
                "wp": Wp2,
                "wasad": wasad2,
                "casad": casad2,
                "biasx": biasx2,
                "eidx": eidx[c],
                "batchf": bf,
            }
        )
    r2 = _run(nc2, maps2, ["pooled"], sim=_sim)

    pool_sum = np.sum([r2[c]["pooled"] for c in range(NC)], axis=0)[:G]
    cnt = np.bincount(batch, minlength=G).astype(np.float32)
    pooled = pool_sum / np.maximum(cnt, 1.0)[:, None]
    return (pooled @ fc_w + fc_b).astype(np.float32)


# revision 21
# speedup vs baseline: 1.2076x; 1.0743x over previous
"""GAT message-passing network (2x GATConv + BN + global mean pool) on 8 trn2 cores.

Two launches (one per GAT layer; layer-1 shard outputs are gathered on the
host between them — the on-device AllGather path crashes NRT through axon).
Sharding: dst-nodes partitioned across cores (6250/core); each core processes
the edges whose dst lands in its shard (edges sorted by dst, padded so every
128-edge tile maps into a single 128-node block). BatchNorms are folded into
the GAT weights on the host (they are affine in eval mode). A dense phase
builds a concatenated node table [x | alo | ahi]; per edge tile the kernel
gathers src/dst rows from it by indirect DMA, computes per-edge features with
a dense matmul, and does segment softmax/aggregation with one-hot
selection-matrix matmuls accumulating in PSUM; the softmax denominator is
divided out after aggregation (it is constant within a segment). Graph-mean
pooling partials come from batch-id one-hot matmuls; the final tiny FC runs
on the host as part of unsharding.
"""

import sys

import numpy as np

for _p in ("/opt/trn_rl_repo",):
    if _p not in sys.path:
        sys.path.insert(0, _p)

import concourse.bass as bass
import concourse.tile as tile
from concourse import bacc, mybir
from concourse.bass import IndirectOffsetOnAxis, ts
from concourse.masks import make_identity

P = 128
FP = mybir.dt.float32
I32 = mybir.dt.int32
AF = mybir.ActivationFunctionType
OP = mybir.AluOpType


def _ceil(a, b):
    return -(-a // b)


def _emit_dense(nc, tc, sfx, x_ap, xc_ap, ww_sb, cb_sb, ident, N, F, H):
    """xcat[n, :] = [x[n] | alo | ahi], alo/ahi = x[n] @ wasad + casad."""
    NT_full = N // P
    GW = 4
    with (
        tc.tile_pool(name=f"dn{sfx}", bufs=3) as dn,
        tc.tile_pool(name=f"dnp{sfx}", bufs=2, space="PSUM") as dnp,
    ):
        groups = [
            list(range(g, min(g + GW, NT_full))) for g in range(0, NT_full, GW)
        ]
        for grp in groups:
            gw = len(grp)
            r0 = grp[0] * P
            xg4 = dn.tile([P, GW * F], FP, tag="xg", name="xg4")
            nc.sync.dma_start(
                xg4[:, : gw * F].rearrange("p (t f) -> p t f", f=F),
                x_ap[r0 : r0 + gw * P, :].rearrange("(t p) f -> p t f", p=P),
            )
            xts4 = dn.tile([F, GW * P], FP, tag="xts", name="xts4")
            xt_ps4 = dnp.tile([F, GW * P], FP, tag="t1", space="PSUM", name="xt_ps4")
            for j in range(gw):
                nc.tensor.transpose(
                    xt_ps4[:, j * P : (j + 1) * P],
                    xg4[:, j * F : (j + 1) * F],
                    ident[:],
                )
            nc.scalar.copy(xts4[:, : gw * P], xt_ps4[:, : gw * P])
            aa_ps = dnp.tile(
                [2 * H, GW * P], FP, tag="t2", space="PSUM", name="aa_ps"
            )
            nc.tensor.matmul(
                aa_ps[:, : gw * P],
                lhsT=ww_sb[:],
                rhs=xts4[:, : gw * P],
                start=True,
                stop=True,
            )
            aab = dn.tile([2 * H, GW * P], FP, tag="aab", name="aab")
            nc.scalar.activation(
                aab[:, : gw * P], aa_ps[:, : gw * P], AF.Identity, bias=cb_sb[:]
            )
            aaTs = dn.tile([P, GW * 2 * H], FP, tag="aats", name="aaTs")
            aaT_ps4 = dnp.tile(
                [P, GW * 2 * H], FP, tag="t3", space="PSUM", name="aaT_ps4"
            )
            for j in range(gw):
                nc.tensor.transpose(
                    aaT_ps4[:, j * 2 * H : (j + 1) * 2 * H],
                    aab[:, j * P : (j + 1) * P],
                    ident[: 2 * H, : 2 * H],
                )
            nc.vector.tensor_copy(aaTs[:, : gw * 2 * H], aaT_ps4[:, : gw * 2 * H])
            nc.sync.dma_start(
                xc_ap[r0 : r0 + gw * P, :F].rearrange("(t p) f -> p t f", p=P),
                xg4[:, : gw * F].rearrange("p (t f) -> p t f", f=F),
            )
            nc.scalar.dma_start(
                xc_ap[r0 : r0 + gw * P, F:].rearrange("(t p) h -> p t h", p=P),
                aaTs[:, : gw * 2 * H].rearrange("p (t h) -> p t h", h=2 * H),
            )
        if N % P:
            i = NT_full
            p = N - i * P
            xg = dn.tile([P, F], FP, tag="xgp", name="xgp")
            nc.vector.memset(xg[:], 0.0)
            nc.sync.dma_start(xg[:p], x_ap[i * P : i * P + p, :])
            xt_ps = dnp.tile([F, P], FP, tag="t1", space="PSUM", name="xt_psp")
            nc.tensor.transpose(xt_ps[:], xg[:], ident[:])
            xts = dn.tile([F, P], FP, tag="xtsp", name="xtsp")
            nc.vector.tensor_copy(xts[:], xt_ps[:])
            aa_ps = dnp.tile([2 * H, P], FP, tag="t2", space="PSUM", name="aa_psp")
            nc.tensor.matmul(
                aa_ps[:], lhsT=ww_sb[:], rhs=xts[:], start=True, stop=True
            )
            aab = dn.tile([2 * H, P], FP, tag="aabp", name="aabp")
            nc.scalar.activation(aab[:], aa_ps[:], AF.Identity, bias=cb_sb[:])
            aaT_ps = dnp.tile([P, 2 * H], FP, tag="t3", space="PSUM", name="aaT_psp")
            nc.tensor.transpose(aaT_ps[:], aab[:], ident[: 2 * H, : 2 * H])
            aaTs = dn.tile([P, 2 * H], FP, tag="aatsp", name="aaTsp")
            nc.vector.tensor_copy(aaTs[:], aaT_ps[:])
            nc.sync.dma_start(xc_ap[i * P : i * P + p, :F], xg[:p])
            nc.scalar.dma_start(xc_ap[i * P : i * P + p, F:], aaTs[:p])


def _emit_edges(
    nc,
    tc,
    sfx,
    xc_ap,
    ei_ap,
    wp_sb,
    bx_sb,
    ident,
    iotaf,
    F,
    H,
    C,
    NSH,
    TBB,
    NB,
    ho_ap=None,
    pool_cfg=None,
):
    """Edge pass: attention + weighted aggregation per 128-node block.
    Writes block outputs to ho_ap [NSH, C] if given; accumulates graph-pool
    partials if pool_cfg=(bf_ap, po_ap, GCH)."""
    HC = H * C
    hpc = max(1, min(H, 480 // C))
    chunks = [(h0, min(h0 + hpc, H)) for h0 in range(0, H, hpc)]
    EB = (chunks[0][1] - chunks[0][0]) * C
    ycol = [0]
    for k, (h0, h1) in enumerate(chunks):
        ycol.append(ycol[-1] + (h1 - h0) * C + (H if k == 0 else 0))

    with (
        tc.tile_pool(name=f"eg{sfx}", bufs=4) as eg,
        tc.tile_pool(name=f"egp_t{sfx}", bufs=2, space="PSUM") as egp_t,
        tc.tile_pool(name=f"egp_xt{sfx}", bufs=2, space="PSUM") as egp_xt,
        tc.tile_pool(name=f"egp_agg{sfx}", bufs=2, space="PSUM") as egp_agg,
        tc.tile_pool(name=f"egp_pool{sfx}", bufs=1, space="PSUM") as egp_pool,
    ):
        if pool_cfg:
            bf_ap, po_ap, GCH = pool_cfg
            pool_ps = [
                egp_pool.tile(
                    [P, C], FP, tag=f"pool{g}", space="PSUM", name=f"pool{g}"
                )
                for g in range(GCH)
            ]
        cur = None
        offs = [0]
        for v in TBB:
            offs.append(offs[-1] + v)
        for b in range(NB):
          TB = TBB[b]
          for j in range(TB):
            t = offs[b] + j
            et = eg.tile([P, 3], I32, tag="et", name="et")
            nc.scalar.dma_start(et[:], ei_ap[ts(t, P)])
            dlf = eg.tile([P, 1], FP, tag="dlf", name="dlf")
            nc.vector.tensor_copy(dlf[:], et[:, 2:3])
            xsf = eg.tile([P, F + 2 * H], FP, tag="xsf", name="xsf")
            nc.gpsimd.indirect_dma_start(
                out=xsf[:],
                out_offset=None,
                in_=xc_ap[:, :],
                in_offset=IndirectOffsetOnAxis(ap=et[:, 0:1], axis=0),
            )
            gdf = eg.tile([P, F + 2 * H], FP, tag="gdf", name="gdf")
            nc.gpsimd.indirect_dma_start(
                out=gdf[:],
                out_offset=None,
                in_=xc_ap[:, :],
                in_offset=IndirectOffsetOnAxis(ap=et[:, 1:2], axis=0),
            )
            y = eg.tile([P, HC + H], FP, tag="y", name="y")
            pre = eg.tile([P, H], FP, tag="pre", name="pre")
            nc.vector.tensor_tensor(
                out=pre[:],
                in0=xsf[:, F : F + H],
                in1=gdf[:, F + H : F + 2 * H],
                op=OP.add,
            )
            tmp = eg.tile([P, H], FP, tag="tmp", name="tmp")
            nc.vector.tensor_scalar_mul(tmp[:], pre[:], 0.2)
            prl = eg.tile([P, H], FP, tag="prl", name="prl")
            nc.vector.tensor_tensor(prl[:], pre[:], tmp[:], op=OP.max)
            nc.scalar.activation(y[:, EB : EB + H], prl[:], AF.Exp)
            st = eg.tile([P, P], FP, tag="st", name="st")
            nc.vector.tensor_scalar(st[:], iotaf[:], dlf[:, :1], None, OP.is_equal)
            tp = egp_t.tile([F, P], FP, tag="tp", space="PSUM", name="tp")
            nc.tensor.transpose(tp[:], xsf[:, :F], ident[:])
            xts = eg.tile([F, P], FP, tag="xts2", name="xts2")
            nc.scalar.copy(xts[:], tp[:])
            if j == 0:
                cur = [
                    egp_agg.tile(
                        [P, ycol[k + 1] - ycol[k]],
                        FP,
                        tag=f"agg{k}",
                        space="PSUM",
                        name=f"agg{k}",
                    )
                    for k in range(len(chunks))
                ]
            aggs = cur
            for k, (h0, h1) in enumerate(chunks):
                hh = h1 - h0
                xtp = egp_xt.tile(
                    [P, hh * C], FP, tag="xtp", space="PSUM", name="xtp"
                )
                nc.tensor.matmul(
                    xtp[:],
                    lhsT=xts[:],
                    rhs=wp_sb[:, h0 * C : h1 * C],
                    start=True,
                    stop=True,
                )
                yc = ycol[k]
                nc.vector.tensor_tensor(
                    out=y[:, yc : yc + hh * C].rearrange("p (h c) -> p h c", c=C),
                    in0=xtp[:].rearrange("p (h c) -> p h c", c=C),
                    in1=y[:, EB + h0 : EB + h1, None].to_broadcast([P, hh, C]),
                    op=OP.mult,
                )
                nc.tensor.matmul(
                    aggs[k][:],
                    lhsT=st[:],
                    rhs=y[:, ycol[k] : ycol[k + 1]],
                    start=(j == 0),
                    stop=(j == TB - 1),
                )
            if j == TB - 1:
                s1 = eg.tile([P, H], FP, tag="s1", name="s1")
                nc.vector.tensor_scalar_add(s1[:], aggs[0][:, EB : EB + H], 1e-16)
                siv = eg.tile([P, H], FP, tag="siv", name="siv")
                nc.vector.reciprocal(siv[:], s1[:])
                t1 = eg.tile([P, HC], FP, tag="t1f", name="t1f")
                for k, (h0, h1) in enumerate(chunks):
                    hh = h1 - h0
                    nc.vector.tensor_tensor(
                        out=t1[:, h0 * C : h1 * C].rearrange(
                            "p (h c) -> p h c", c=C
                        ),
                        in0=aggs[k][:, : hh * C].rearrange("p (h c) -> p h c", c=C),
                        in1=siv[:, h0:h1, None].to_broadcast([P, hh, C]),
                        op=OP.mult,
                    )
                t2 = eg.tile([P, C], FP, tag="t2f", name="t2f")
                nc.vector.reduce_sum(
                    t2[:, :, None],
                    t1[:].rearrange("p (h c) -> p c h", c=C),
                    axis=mybir.AxisListType.X,
                )
                t3 = eg.tile([P, C], FP, tag="t3f", name="t3f")
                nc.vector.tensor_tensor(t3[:], t2[:], bx_sb[:], op=OP.add)
                u = eg.tile([P, C], FP, tag="uf", name="uf")
                nc.vector.tensor_scalar_mul(u[:], t3[:], 1.0 / H)
                v = eg.tile([P, C], FP, tag="vf", name="vf")
                nc.vector.tensor_scalar_mul(v[:], u[:], 0.01)
                ho = eg.tile([P, C], FP, tag="hof", name="hof")
                nc.vector.tensor_tensor(ho[:], u[:], v[:], op=OP.max)
                valid = min(P, NSH - b * P)
                if ho_ap is not None:
                    nc.scalar.dma_start(ho_ap[b * P : b * P + valid, :], ho[:valid])
                if pool_cfg:
                    bf = eg.tile([P, 1], FP, tag="bf", name="bf")
                    nc.sync.dma_start(bf[:], bf_ap[ts(b, P)])
                    for g in range(GCH):
                        bs = eg.tile([P, 1], FP, tag="bs", name="bs")
                        nc.vector.tensor_scalar(
                            bs[:], bf[:], float(g * P), None, OP.subtract
                        )
                        sb = eg.tile([P, P], FP, tag="sb", name="sb")
                        nc.vector.tensor_scalar(
                            sb[:], iotaf[:], bs[:, :1], None, OP.is_equal
                        )
                        nc.tensor.matmul(
                            pool_ps[g][:],
                            lhsT=sb[:],
                            rhs=ho[:],
                            start=(b == 0),
                            stop=(b == NB - 1),
                        )
        if pool_cfg:
            for g in range(GCH):
                pc = eg.tile([P, C], FP, tag="pc", name="pc")
                nc.vector.tensor_copy(pc[:], pool_ps[g][:])
                nc.sync.dma_start(po_ap[ts(g, P)], pc[:])


def build_layer_program(N, F, H, C, NSH, TBB, NB, Ep, G=None):
    """One GAT layer (+ optional graph pooling) over a dst-node shard."""
    pool = G is not None
    GCH = _ceil(G, P) if pool else 0
    nc = bacc.Bacc("TRN2", target_bir_lowering=False, debug=False)

    x_d = nc.dram_tensor("x", [N, F], FP, kind="ExternalInput").ap()
    wp_d = nc.dram_tensor("wp", [F, H * C], FP, kind="ExternalInput").ap()
    ww_d = nc.dram_tensor("wasad", [F, 2 * H], FP, kind="ExternalInput").ap()
    cb_d = nc.dram_tensor("casad", [2 * H, 1], FP, kind="ExternalInput").ap()
    bx_d = nc.dram_tensor("biasx", [P, C], FP, kind="ExternalInput").ap()
    ei_d = nc.dram_tensor("eidx", [Ep, 3], I32, kind="ExternalInput").ap()
    if pool:
        bf_d = nc.dram_tensor("batchf", [NB * P, 1], FP, kind="ExternalInput").ap()
        po_d = nc.dram_tensor("pooled", [GCH * P, C], FP, kind="ExternalOutput").ap()
    ho_d = nc.dram_tensor("hout", [NSH, C], FP, kind="ExternalOutput").ap()
    xc_d = nc.dram_tensor("xcat", [N, F + 2 * H], FP).ap()

    with tile.TileContext(nc) as tc:
        with tc.tile_pool(name="const", bufs=1) as const:
            ident = const.tile([P, P], FP)
            make_identity(nc, ident[:])
            iota_i = const.tile([P, P], I32)
            nc.gpsimd.iota(iota_i[:], pattern=[[1, P]], base=0, channel_multiplier=0)
            iotaf = const.tile([P, P], FP)
            nc.vector.tensor_copy(iotaf[:], iota_i[:])

            def load_const(name, ap_, shape):
                sb = const.tile(shape, FP, name=name)
                nc.sync.dma_start(sb[:], ap_[:, :])
                return sb

            wp_sb = load_const("wp_sb", wp_d, [F, H * C])
            ww_sb = load_const("ww_sb", ww_d, [F, 2 * H])
            cb_sb = load_const("cb_sb", cb_d, [2 * H, 1])
            bx_sb = load_const("bx_sb", bx_d, [P, C])

            _emit_dense(nc, tc, "1", x_d, xc_d, ww_sb, cb_sb, ident, N, F, H)
            _emit_edges(
                nc, tc, "1", xc_d, ei_d, wp_sb, bx_sb, ident, iotaf,
                F, H, C, NSH, TBB, NB, ho_ap=ho_d,
                pool_cfg=(bf_d, po_d, GCH) if pool else None,
            )
    nc.compile()
    return nc


def _fold(W, a_s, a_d, b, g, bb, m, v, H, C):
    """Fold eval-mode BatchNorm (scale s, shift t) into GAT weights."""
    F = W.shape[0]
    s = g / np.sqrt(v + 1e-5)
    t = bb - m * s
    Wp = (s[:, None] * W).astype(np.float32)
    cxt = t @ W  # [H*C]
    Wr = Wp.reshape(F, H, C)
    was = np.einsum("fhc,hc->fh", Wr, a_s)
    wad = np.einsum("fhc,hc->fh", Wr, a_d)
    cas = np.einsum("hc,hc->h", cxt.reshape(H, C), a_s)
    cad = np.einsum("hc,hc->h", cxt.reshape(H, C), a_d)
    bias_true = b + cxt.reshape(H, C).mean(0)  # [C]
    wasad = np.concatenate([was, wad], 1).astype(np.float32)
    casad = np.concatenate([cas, cad])[:, None].astype(np.float32)
    biasx = np.broadcast_to(H * bias_true, (P, C)).astype(np.float32).copy()
    return Wp, wasad, casad, biasx


def _prep_edges(src, dst, N, NC, NSH):
    """Sort edges by dst, shard by dst-node, pad each 128-node block to a
    uniform number of 128-edge tiles (TB). Pads have dstloc=200 -> zero
    one-hot column -> inert. Returns eidx [NC, Ep, 3] int32 =
    (src row, dst row, block-local dst)."""
    order = np.argsort(dst, kind="stable")
    ss = src[order].astype(np.int64)
    ds = dst[order].astype(np.int64)
    core = ds // NSH
    loc = ds - core * NSH
    NB = _ceil(NSH, P)
    blk = loc // P
    gid = core * NB + blk
    cnt = np.bincount(gid, minlength=NC * NB)
    # per-block tile count: max over cores for that block (SPMD-uniform)
    tbb = np.maximum(1, -(-cnt.reshape(NC, NB).max(axis=0) // P))
    offs = np.zeros(NB + 1, np.int64)
    offs[1:] = np.cumsum(tbb)
    Ep = int(offs[NB]) * P
    starts = np.zeros(NC * NB, np.int64)
    starts[1:] = np.cumsum(cnt)[:-1]
    rank = np.arange(len(ds)) - starts[gid]
    pos = offs[blk] * P + rank
    eidx = np.zeros((NC, Ep, 3), np.int32)
    eidx[:, :, 2] = 200
    eidx[core, pos, 0] = ss
    eidx[core, pos, 1] = ds
    eidx[core, pos, 2] = (loc - blk * P).astype(np.int32)
    return eidx, [int(v) for v in tbb], NB, Ep


TRACE = False  # test harness sets True to collect HW exec times
LAST_EXEC_NS = []
LAST_PROGRAMS = []  # (nc, in_maps) pairs from the most recent kernel() call


def _run(nc, in_maps, out_names, sim=False):
    if sim:
        from concourse.bass_interp import CoreSim, MultiCoreSim

        ncores = len(in_maps)
        if nc.has_collectives:
            ms = MultiCoreSim(nc, ncores)
            for c, m in enumerate(in_maps):
                for k, val in m.items():
                    ms.cores[c].tensor(k)[:] = val
            ms.simulate()
            return [
                {o: np.array(ms.cores[c].mem_tensor(o)) for o in out_names}
                for c in range(ncores)
            ]
        res = []
        for m in in_maps:
            s = CoreSim(nc)
            for k, val in m.items():
                s.tensor(k)[:] = val
            s.simulate()
            res.append({o: np.array(s.mem_tensor(o)) for o in out_names})
        return res
    from concourse.bass_utils import run_bass_kernel_spmd

    LAST_PROGRAMS.append((nc, in_maps))
    r = run_bass_kernel_spmd(
        nc, in_maps, core_ids=list(range(len(in_maps))), trace=TRACE
    )
    if TRACE:
        LAST_EXEC_NS.append(r.exec_time_ns)
    return r.results


def kernel(
    x,
    edge_index,
    batch,
    bn1_g,
    bn1_b,
    bn1_m,
    bn1_v,
    W1,
    as1,
    ad1,
    b1,
    bn2_g,
    bn2_b,
    bn2_m,
    bn2_v,
    W2,
    as2,
    ad2,
    b2,
    fc_w,
    fc_b,
    _n_cores=8,
    _G=256,
    _sim=False,
):
    x = np.ascontiguousarray(np.asarray(x, np.float32))
    ei = np.asarray(edge_index).astype(np.int64)
    batch = np.asarray(batch).astype(np.int64)
    to32 = lambda a: np.ascontiguousarray(np.asarray(a, np.float32))
    W1, as1, ad1, b1 = to32(W1), to32(as1), to32(ad1), to32(b1)
    W2, as2, ad2, b2 = to32(W2), to32(as2), to32(ad2), to32(b2)
    fc_w, fc_b = to32(fc_w), to32(fc_b)

    N, F = x.shape
    H, C1 = as1.shape
    C2 = as2.shape[1]
    NC = _n_cores
    G = _G
    NSH = N // NC
    assert NSH * NC == N

    src = np.concatenate([ei[0], np.arange(N, dtype=np.int64)])
    dst = np.concatenate([ei[1], np.arange(N, dtype=np.int64)])
    eidx, TBB, NB, Ep = _prep_edges(src, dst, N, NC, NSH)

    Wp1, wasad1, casad1, biasx1 = _fold(
        W1, as1, ad1, b1, bn1_g, bn1_b, bn1_m, bn1_v, H, C1
    )
    Wp2, wasad2, casad2, biasx2 = _fold(
        W2, as2, ad2, b2, bn2_g, bn2_b, bn2_m, bn2_v, H, C2
    )

    # ---- launch 1: GAT layer 1 -> h2 shards ----
    nc1 = build_layer_program(N, F, H, C1, NSH, TBB, NB, Ep)
    maps1 = [
        {
            "x": x,
            "wp": Wp1,
            "wasad": wasad1,
            "casad": casad1,
            "biasx": biasx1,
            "eidx": eidx[c],
        }
        for c in range(NC)
    ]
    r1 = _run(nc1, maps1, ["hout"], sim=_sim)
    h2 = np.concatenate([r1[c]["hout"] for c in range(NC)], 0)

    # ---- launch 2: GAT layer 2 + graph pooling partials ----
    nc2 = build_layer_program(N, C1, H, C2, NSH, TBB, NB, Ep, G=G)
    maps2 = []
    for c in range(NC):
        bf = np.full((NB * P, 1), 1e4, np.float32)
        bf[:NSH, 0] = batch[c * NSH : (c + 1) * NSH]
        maps2.append(
            {
                "x": h2,
                "wp": Wp2,
                "wasad": wasad2,
                "casad": casad2,
                "biasx": biasx2,
                "eidx": eidx[c],
                "batchf": bf,
            }
        )
    r2 = _run(nc2, maps2, ["pooled"], sim=_sim)

    pool_sum = np.sum([r2[c]["pooled"] for c in range(NC)], axis=0)[:G]
    cnt = np.bincount(batch, minlength=G).astype(np.float32)
    pooled = pool_sum / np.maximum(cnt, 1.0)[:, None]
    return (pooled @ fc_w + fc_b).astype(np.float32)
